# revision 26
# baseline (speedup 1.0000x reference)
"""Sparse 2D-sliding-window + global-token attention block on 8 TRN2 NeuronCores.

Strategy: data-parallel over batch (B=8 -> one batch element per core, zero
collectives). Per core, for one [1032, 1024] sequence:

  - tokens reordered host-side: 1024 patches first (8 exact tiles of 128 =
    4 grid rows each), 8 special/CLS tokens last.  Patch q-tile t only
    attends to patch k-tiles {t-1, t, t+1} plus the specials.
  - QKV projection in bf16 (lhsT = X^T tiles, rhs = W^T), RMS-norm + RoPE in
    row layout (norm weights folded into host-precomputed cos/sin tables),
    then PE-transpose of q~/k~ into [d, m] layout for the score matmuls.
  - scores computed transposed (S^T = K~ Q~^T) into two 3-bank PSUM
    "canvases" per head; the sparsity mask is baked in ADDITIVELY (0/-1e4)
    by an identity-weight matmul that also initializes each bank, so one
    wide scalar-engine Exp per canvas produces masked probabilities
    directly (softmax needs no max-subtraction: RMS-normed rows have L2
    norm exactly 8, so |s| <= 8 and exp(s/8) is safe).  V carries an
    appended ones-column so denominators fall out of the PV matmul as row
    64 of O^T; reciprocals are taken by the vector engine straight off
    PSUM row 64 and broadcast across partitions with a rank-1 matmul.
  - score/PV stages are software-pipelined across heads (scores of head
    h+1 are emitted before PV of head h) to keep the PE busy and the HAM
    clock-gate at full rate.
  - out-projection consumes O^T directly as lhsT (no O transpose needed).
"""

import numpy as np
import ml_dtypes

B, N, DIM, HEADS, HD = 8, 1032, 1024, 16, 64
SPECIAL, GRID, WINDOW = 8, 32, 3
NP = 1024          # patch tokens
P = 128
NT = NP // P       # 8 patch tiles (4 grid rows each)
NC_ = DIM // P     # 8 contraction chunks
EPS = 1e-6
NEG = -1.0e4       # additive mask value; exp(NEG/8) == 0 in bf16
CW = 1536          # canvas width (3 PSUM banks)
SPQ = 1408         # specials-q block offset within canvas half 1
bf16 = ml_dtypes.bfloat16

# ---- band geometry -------------------------------------------------------
# canvas half H in {0,1} holds k-tiles s = 4H..4H+3; window of k-tile s
# covers q-tiles t_lo..t_hi contiguously at canvas offset ws.
GEOM = []
for _H in (0, 1):
    _ws = 0
    for _s in range(4 * _H, 4 * _H + 4):
        _tlo, _thi = max(0, _s - 1), min(NT - 1, _s + 1)
        _w = P * (_thi - _tlo + 1)
        GEOM.append((_H, _s, _ws, _w, _tlo, _thi))
        _ws += _w

# score matmul pieces per half: (s, a, b, qa, stop) -> canvas[:, a:b] +=
# K_s^T Q[:, qa:qa+(b-a)]; `stop` marks the last accumulating matmul of a
# bank (bank2 of half 1 is closed later by the special-special matmul).
SCORE_PIECES = {0: [], 1: []}
for (_H, _s, _ws, _w, _tlo, _thi) in GEOM:
    _q0 = P * _tlo
    _a = _ws
    while _a < _ws + _w:
        _b = min(_ws + _w, (_a // 512 + 1) * 512)
        SCORE_PIECES[_H].append([_s, _a, _b, _q0 + (_a - _ws)])
        _a = _b
for _H in (0, 1):
    _last = {}
    for _idx, (_s, _a, _b, _qa) in enumerate(SCORE_PIECES[_H]):
        _last[_a // 512] = _idx
    for _bk, _idx in _last.items():
        _stop = not (_H == 1 and _bk == 2)  # bank2/half1 closed by ss matmul
        SCORE_PIECES[_H][_idx] = SCORE_PIECES[_H][_idx] + [_stop]
    for _p in SCORE_PIECES[_H]:
        if len(_p) == 4:
            _p.append(False)

# PV pieces per output bank b: (s, half, oa, ob, ra) ->
# po_b[:, oa:ob] += V_s^T P^T(canvas[half][:, ra:ra+(ob-oa)])
PV_PIECES = {0: [], 1: []}
for _b in (0, 1):
    for (_H, _s, _ws, _w, _tlo, _thi) in GEOM:
        _t0, _t1 = max(4 * _b, _tlo), min(4 * _b + 3, _thi)
        if _t0 > _t1:
            continue
        _oa = P * (_t0 - 4 * _b)
        _ob = P * (_t1 + 1 - 4 * _b)
        _ra = _ws + P * (_t0 - _tlo)
        PV_PIECES[_b].append((_s, _H, _oa, _ob, _ra))

_COMPILED = None


def _build():
    from contextlib import ExitStack
    import concourse.bass as bass
    import concourse.tile as tile
    from concourse import bacc, mybir
    from concourse.masks import make_identity

    dt = mybir.dt
    AF = mybir.ActivationFunctionType
    MUL = mybir.AluOpType.mult
    ADD = mybir.AluOpType.add

    nc = bacc.Bacc()

    xT = nc.declare_dram_parameter("xT", [P, NC_, N], dt.bfloat16, isOutput=False)
    wqkv = nc.declare_dram_parameter("wqkv", [P, NC_, 3 * DIM], dt.bfloat16, isOutput=False)
    wo = nc.declare_dram_parameter("wo", [P, NC_, DIM], dt.bfloat16, isOutput=False)
    # folded (norm-weight x cos/sin) tables, reordered to the m-layout, [128, 9, 64]
    cosq = nc.declare_dram_parameter("cosq", [P, NT + 1, HD], dt.bfloat16, isOutput=False)
    sinq = nc.declare_dram_parameter("sinq", [P, NT + 1, HD], dt.bfloat16, isOutput=False)
    cosk = nc.declare_dram_parameter("cosk", [P, NT + 1, HD], dt.bfloat16, isOutput=False)
    sink = nc.declare_dram_parameter("sink", [P, NT + 1, HD], dt.bfloat16, isOutput=False)
    mskc0 = nc.declare_dram_parameter("mskc0", [P, CW], dt.bfloat16, isOutput=False)
    mskc1 = nc.declare_dram_parameter("mskc1", [P, CW], dt.bfloat16, isOutput=False)
    out = nc.declare_dram_parameter("out", [N, DIM], dt.float32, isOutput=True)

    # m-tile geometry: tiles 0..7 are patches (128 rows), tile 8 is specials (8)
    def mslice(i):
        return slice(i * P, i * P + (P if i < NT else SPECIAL))

    def mp(i):
        return P if i < NT else SPECIAL

    with ExitStack() as ctx:
        ctx.enter_context(nc.allow_low_precision(reason="bf16 compute validated against f32 reference"))
        tc = ctx.enter_context(tile.TileContext(nc))
        persist = ctx.enter_context(tc.tile_pool(name="persist", bufs=1))

        # ---- resident SBUF tensors -------------------------------------
        ident = persist.tile([P, P], dt.bfloat16, tag="ident")
        make_identity(nc, ident[:])

        xT_sb = persist.tile([P, NC_, N], dt.bfloat16)
        wq_sb = persist.tile([P, NC_, 3 * DIM], dt.bfloat16)
        wo_sb = persist.tile([P, NC_, DIM], dt.bfloat16)
        tab = {}
        for nm in ("cosq", "sinq", "cosk", "sink"):
            tab[nm] = persist.tile([P, NT + 1, HD], dt.bfloat16, tag=f"tab_{nm}", name=f"tab_{nm}")
        msk_sb = [
            persist.tile([P, CW], dt.bfloat16, tag=f"mskc{_h}", name=f"mskc{_h}_sb")
            for _h in range(2)
        ]
        for c in range(NC_):
            nc.sync.dma_start(xT_sb[:, c, :], xT[:, c, :])
            nc.sync.dma_start(wq_sb[:, c, 0:1536], wqkv[:, c, 0:1536])
            nc.sync.dma_start(wq_sb[:, c, 1536:3072], wqkv[:, c, 1536:3072])
            if c == 3:
                for nm, ap in (("cosq", cosq), ("sinq", sinq), ("cosk", cosk), ("sink", sink)):
                    nc.sync.dma_start(tab[nm][:], ap[:])
        nc.sync.dma_start(msk_sb[0][:], mskc0[:])
        nc.sync.dma_start(msk_sb[1][:], mskc1[:])

        # q~^T stored one head per 128-partition slot with the other head's
        # 64 rows ZERO, so score matmuls contract over the full 128 partitions
        # (k=128 keeps PE-array utilization high -> HAM stays at full clock):
        # lhsT = kT_sb[:, ch, tile] holds the head PAIR, the zeros in qTz kill
        # the other head's contribution.
        qTz = persist.tile([P, HEADS, N], dt.bfloat16, tag="qTz")
        nc.gpsimd.memset(qTz[:], 0.0)
        kT_sb = persist.tile([P, NC_, N], dt.bfloat16, tag="kT")
        # normalized O^T overwrites kT_sb per head (kT for head pair (2c,2c+1)
        # is last read by scores of head 2c+1; norm trails by 2 pipeline
        # stages, so the overwrite is safe) -- saves 16.5KB/partition of SBUF
        oT_sb = kT_sb
        # V with an interleaved ones column: [128, 9 m-tiles, 16 heads, 65]
        v_sb = persist.tile([P, NT + 1, HEADS, HD + 1], dt.bfloat16, tag="v")
        nc.vector.memset(v_sb[:, :, :, HD : HD + 1], 1.0)

        eps_sb = persist.tile([P, 1], dt.float32, tag="eps")
        nc.vector.memset(eps_sb[:], EPS)

        # ---- phase A: QKV projection + RMS norm + RoPE + transpose -----
        with tc.tile_pool(name="psumA", bufs=2, space="PSUM") as psumA, \
             tc.tile_pool(name="sbufA", bufs=2) as sbA:
            # HAM warmup: keep the PE busy while the first DMAs land so the
            # clock-gate reaches 8/8 before the real matmuls start.
            warm = psumA.tile([P, 512], dt.float32, tag="tr", name="warm")
            for _w in range(36):
                nc.tensor.matmul(warm[:P, 0:P], ident[:], ident[:], start=True, stop=True)

            rope_pending = []

            def flush_transposes():
                for (ii, rope, which) in rope_pending:
                    mm = mp(ii)
                    mss = mslice(ii)
                    for half in (0, 1):
                        ptr = psumA.tile([P, 512], dt.bfloat16, tag="tr", name=f"tr{ii}_{half}")
                        for c2 in range(4):
                            cc = 4 * half + c2
                            nc.tensor.transpose(
                                ptr[:P, c2 * P : c2 * P + mm],
                                rope[:mm, cc * P : (cc + 1) * P],
                                ident[:mm, :mm],
                            )
                        src = ptr[:P, :].rearrange("p (c f) -> p c f", c=4)[:, :, :mm]
                        if which == "k":
                            nc.vector.tensor_copy(
                                kT_sb[:, 4 * half : 4 * half + 4, mss], src
                            )
                        else:
                            # d-chunk cc covers heads (2cc, 2cc+1): rows 0:64 of
                            # the transpose are head 2cc, rows 64:128 head 2cc+1
                            nc.vector.tensor_copy(
                                qTz[0:HD, 8 * half : 8 * half + 8 : 2, mss],
                                src[0:HD],
                            )
                            nc.vector.tensor_copy(
                                qTz[HD:P, 8 * half + 1 : 8 * half + 8 : 2, mss],
                                src[HD:P],
                            )
                rope_pending.clear()

            for i in [NT] + list(range(NT)):
                m = mp(i)
                ms = mslice(i)
                psA = psumA.tile([P, 1536], dt.float32, tag="qkv", name=f"psA{i}")
                for c in range(NC_):
                    lhsT = xT_sb[:, c, ms]
                    for j in range(3):
                        nc.tensor.matmul(
                            psA[:m, j * 512 : (j + 1) * 512],
                            lhsT,
                            wq_sb[:, c, j * 512 : (j + 1) * 512],
                            start=(c == 0),
                            stop=(c == NC_ - 1),
                        )
                    if i == NT:
                        # fill the DMA-gated startup gaps with warmup matmuls
                        # so the HAM clock-gate stays at full rate
                        for _w in range(4):
                            nc.tensor.matmul(warm[:P, 0:P], ident[:], ident[:], start=True, stop=True)
                psB = psumA.tile([P, 1536], dt.float32, tag="qkv", name=f"psB{i}")
                for c in range(NC_):
                    lhsT = xT_sb[:, c, ms]
                    for j in range(3, 6):
                        nc.tensor.matmul(
                            psB[:m, (j - 3) * 512 : (j - 2) * 512],
                            lhsT,
                            wq_sb[:, c, j * 512 : (j + 1) * 512],
                            start=(c == 0),
                            stop=(c == NC_ - 1),
                        )
                    if i == NT:
                        for _w in range(4):
                            nc.tensor.matmul(warm[:P, 0:P], ident[:], ident[:], start=True, stop=True)
                flush_transposes()
                # V: copy into interleaved [head, 65] layout (one wide ACT)
                nc.scalar.copy(
                    v_sb[:m, i, :, 0:HD],
                    psB[:m, 512:1536].rearrange("p (h d) -> p h d", h=HEADS),
                )
                # Q and K: norm + rope
                for which, (j0, cosn, sinn) in (
                    ("q", (0, "cosq", "sinq")),
                    ("k", (2, "cosk", "sink")),
                ):
                    raw = sbA.tile([P, DIM], dt.bfloat16, tag="raw")
                    if which == "q":
                        nc.scalar.copy(raw[:m, 0:1024], psA[:m, 0:1024])
                    else:
                        nc.scalar.copy(raw[:m, 0:512], psA[:m, 1024:1536])
                        nc.scalar.copy(raw[:m, 512:1024], psB[:m, 0:512])
                    sq = sbA.tile([P, DIM], dt.bfloat16, tag="tsin")
                    nc.gpsimd.tensor_tensor(sq[:m], raw[:m], raw[:m], op=MUL)
                    ssum = sbA.tile([P, HEADS], dt.float32, tag="ssum", bufs=3)
                    nc.vector.reduce_sum(
                        ssum[:m],
                        sq[:m].rearrange("p (h d) -> p h d", h=HEADS),
                        axis=mybir.AxisListType.X,
                    )
                    rstd = sbA.tile([P, HEADS], dt.float32, tag="rstd", bufs=3)
                    nc.scalar.activation(rstd[:m], ssum[:m], AF.Sqrt, bias=eps_sb[:m], scale=1.0 / HD)
                    rst = sbA.tile([P, HEADS], dt.bfloat16, tag="rst", bufs=3)
                    nc.vector.reciprocal(rst[:m], rstd[:m])
                    rv = raw[:m].rearrange("p (h two half) -> p h two half", h=HEADS, two=2)
                    cosw = tab[cosn][:m, i, None, :].to_broadcast((m, HEADS, HD))
                    tc_t = sbA.tile([P, DIM], dt.bfloat16, tag="tcos")
                    nc.vector.tensor_tensor(
                        tc_t[:m].rearrange("p (h d) -> p h d", h=HEADS),
                        raw[:m].rearrange("p (h d) -> p h d", h=HEADS),
                        cosw,
                        op=MUL,
                    )
                    ts_t = sbA.tile([P, DIM], dt.bfloat16, tag="tsin")
                    tsv = ts_t[:m].rearrange("p (h two half) -> p h two half", h=HEADS, two=2)
                    sin4 = (
                        tab[sinn][:m, i, None, :]
                        .rearrange("p o (two half) -> p o two half", two=2)
                        .to_broadcast((m, HEADS, 2, HD // 2))
                    )
                    nc.vector.tensor_tensor(tsv[:, :, :, :], rv[:, :, ::-1, :], sin4, op=MUL)
                    nc.vector.tensor_tensor(tc_t[:m], tc_t[:m], ts_t[:m], op=ADD)
                    rope = sbA.tile([P, DIM], dt.bfloat16, tag="rope", bufs=4)
                    nc.vector.tensor_tensor(
                        rope[:m].rearrange("p (h d) -> p h d", h=HEADS),
                        tc_t[:m].rearrange("p (h d) -> p h d", h=HEADS),
                        rst[:m, :, None].to_broadcast((m, HEADS, HD)),
                        op=MUL,
                    )
                    rope_pending.append((i, rope, which))

            flush_transposes()
            for c in range(NC_):
                nc.sync.dma_start(wo_sb[:, c, :], wo[:, c, :])

        # ---- phase B: banded attention, per head, software-pipelined ---
        with tc.tile_pool(name="psumB", bufs=2, space="PSUM") as psumB, \
             tc.tile_pool(name="sbufB", bufs=2) as sbB:

            def emit_scores(h):
                pb = HD * (h % 2)
                ch = h // 2
                qTh = qTz[:, h, :]          # [128, N], other head's rows zero
                kTh = kT_sb[:, ch, :]       # [128, N], head pair packed
                # special-k scores S^T[sk, q] = [8, 1024]
                spk = psumB.tile([P, 1024], dt.float32, tag="canvas", name=f"spk{h}")
                for jj in range(2):
                    nc.tensor.matmul(
                        spk[:SPECIAL, jj * 512 : (jj + 1) * 512],
                        kTh[:, NP : NP + SPECIAL],
                        qTh[:, jj * 512 : (jj + 1) * 512],
                        start=True,
                        stop=True,
                    )
                ptk = sbB.tile([P, 1024], dt.bfloat16, tag="ptk", bufs=2, name=f"ptk{h}")
                nc.scalar.activation(ptk[:SPECIAL, :], spk[:SPECIAL, :], AF.Exp, scale=0.125)
                ptses = []
                for half in (0, 1):
                    cv = psumB.tile([P, CW], dt.float32, tag="canvas", name=f"cv{h}_{half}")
                    # additive mask initializes all 3 banks (start=True)
                    for bk in range(3):
                        nc.tensor.matmul(
                            cv[:, bk * 512 : (bk + 1) * 512],
                            ident[:],
                            msk_sb[half][:, bk * 512 : (bk + 1) * 512],
                            start=True,
                            stop=False,
                            skip_group_check=True,
                        )
                    for (s, a, b, qa, is_stop) in SCORE_PIECES[half]:
                        nc.tensor.matmul(
                            cv[:, a:b],
                            kTh[:, s * P : (s + 1) * P],
                            qTh[:, qa : qa + (b - a)],
                            start=False,
                            stop=is_stop,
                            skip_group_check=True,
                        )
                    if half == 1:
                        for s in range(NT):
                            nc.tensor.matmul(
                                cv[:, SPQ + 8 * s : SPQ + 8 * s + 8],
                                kTh[:, s * P : (s + 1) * P],
                                qTh[:, NP : NP + SPECIAL],
                                start=False,
                                stop=False,
                                skip_group_check=True,
                            )
                        nc.tensor.matmul(
                            cv[:SPECIAL, SPQ + 64 : SPQ + 72],
                            kTh[:, NP : NP + SPECIAL],
                            qTh[:, NP : NP + SPECIAL],
                            start=False,
                            stop=True,
                            skip_group_check=True,
                        )
                    pts = sbB.tile([P, CW], dt.bfloat16, tag="pts", bufs=4, name=f"pts{h}_{half}")
                    nc.scalar.activation(pts[:], cv[:], AF.Exp, scale=0.125)
                    ptses.append(pts)
                return (h, ptses[0], ptses[1], ptk)

            def emit_pv(state):
                h, pts0, pts1, ptk = state
                pb = HD * (h % 2)
                ch = h // 2
                ptsh = (pts0, pts1)
                po = []
                for b2 in (0, 1):
                    pot = psumB.tile([P, 512], dt.float32, tag="pv", name=f"po{h}_{b2}")
                    # full-width specials matmul first: start=True covers the
                    # whole bank so the banded pieces accumulate on written psum
                    nc.tensor.matmul(
                        pot[: HD + 1, 0:512],
                        v_sb[:SPECIAL, NT, h, :],
                        ptk[:SPECIAL, b2 * 512 : (b2 + 1) * 512],
                        start=True,
                        stop=False,
                        skip_group_check=True,
                    )
                    for k2, (s, Hf, oa, ob, ra) in enumerate(PV_PIECES[b2]):
                        nc.tensor.matmul(
                            pot[: HD + 1, oa:ob],
                            v_sb[:, s, h, :],
                            ptsh[Hf][:, ra : ra + (ob - oa)],
                            start=False,
                            stop=(k2 == len(PV_PIECES[b2]) - 1),
                            skip_group_check=True,
                        )
                    po.append(pot)
                po8 = psumB.tile([P, 512], dt.float32, tag="pv", name=f"po8_{h}")
                for s in range(NT):
                    nc.tensor.matmul(
                        po8[: HD + 1, 0:SPECIAL],
                        v_sb[:, s, h, :],
                        pts1[:, SPQ + 8 * s : SPQ + 8 * s + 8],
                        start=(s == 0),
                        stop=False,
                    )
                nc.tensor.matmul(
                    po8[: HD + 1, 0:SPECIAL],
                    v_sb[:SPECIAL, NT, h, :],
                    pts1[:SPECIAL, SPQ + 64 : SPQ + 72],
                    start=False,
                    stop=True,
                )
                # denominators: gather row 64 of the po banks into the `rec`
                # row, DMA-scatter the 1024 band values across 128 partitions,
                # reciprocal on 128 DVE lanes, DMA back into the row.
                rec = sbB.tile([P, 1040], dt.bfloat16, tag="rec", bufs=2, name=f"rec{h}")
                dcol = sbB.tile([P, SPECIAL + 1], dt.bfloat16, tag="dcol", bufs=2, name=f"dc{h}")
                dcolr = sbB.tile([P, SPECIAL + 1], dt.bfloat16, tag="dcolr", bufs=2, name=f"dcr{h}")
                nc.vector.tensor_copy(rec[HD : HD + 1, 0:512], po[0][HD : HD + 1, 0:512])
                nc.vector.tensor_copy(rec[HD : HD + 1, 512:1024], po[1][HD : HD + 1, 0:512])
                nc.vector.tensor_copy(rec[HD : HD + 1, 1024:1032], po8[HD : HD + 1, 0:SPECIAL])
                nc.sync.dma_start(dcol[:, 0:SPECIAL], rec[HD : HD + 1, 0:1024])
                nc.sync.dma_start(dcol[0:SPECIAL, SPECIAL : SPECIAL + 1], rec[HD : HD + 1, 1024:1032])
                nc.vector.reciprocal(dcolr[:, 0:SPECIAL], dcol[:, 0:SPECIAL])
                nc.vector.reciprocal(
                    dcolr[0:SPECIAL, SPECIAL : SPECIAL + 1],
                    dcol[0:SPECIAL, SPECIAL : SPECIAL + 1],
                )
                # scatter 1/den back to partition 0 so gpsimd can broadcast it
                nc.sync.dma_start(rec[0:1, 0:1024], dcolr[:, 0:SPECIAL])
                nc.sync.dma_start(rec[0:1, 1024:1032], dcolr[0:SPECIAL, SPECIAL : SPECIAL + 1])
                ou = []
                for b2 in (0, 1):
                    out_t = sbB.tile([HD, 512], dt.bfloat16, tag="ou", bufs=4, name=f"ou{h}_{b2}")
                    nc.vector.tensor_copy(out_t[:HD, :], po[b2][:HD, 0:512])
                    ou.append(out_t)
                ou8 = sbB.tile([HD, SPECIAL], dt.bfloat16, tag="ou8", bufs=2, name=f"ou8_{h}")
                nc.vector.tensor_copy(ou8[:HD, :SPECIAL], po8[:HD, 0:SPECIAL])
                return (h, rec, ou[0], ou[1], ou8)

            def emit_norm(state):
                h, rec, ou0, ou1, ou8 = state
                pb = HD * (h % 2)
                ch = h // 2
                # broadcast 1/den from partition 0 to all 64 output partitions
                pbc = sbB.tile([HD, 1032], dt.bfloat16, tag="pbc", bufs=2, name=f"pbc{h}")
                nc.gpsimd.partition_broadcast(pbc[:HD, :], rec[0:1, 0:1032])
                for b2 in (0, 1):
                    nc.vector.tensor_tensor(
                        oT_sb[pb : pb + HD, ch, b2 * 512 : (b2 + 1) * 512],
                        (ou0, ou1)[b2][:HD, :],
                        pbc[:HD, b2 * 512 : (b2 + 1) * 512],
                        op=MUL,
                    )
                nc.vector.tensor_tensor(
                    oT_sb[pb : pb + HD, ch, NP : NP + SPECIAL],
                    ou8[:HD, :SPECIAL],
                    pbc[:HD, 1024:1032],
                    op=MUL,
                )

            stage1, stage2 = None, None
            for h in range(HEADS):
                cur = emit_scores(h)
                if stage1 is not None:
                    s2 = emit_pv(stage1)
                    if stage2 is not None:
                        emit_norm(stage2)
                    stage2 = s2
                stage1 = cur
            stage2_last = emit_pv(stage1)
            emit_norm(stage2)
            emit_norm(stage2_last)

        # ---- phase C: out projection -----------------------------------
        with tc.tile_pool(name="psumC", bufs=2, space="PSUM") as psumC, \
             tc.tile_pool(name="sbufC", bufs=2) as sbC:
            for i in [NT] + list(range(NT)):
                m = mp(i)
                row0 = SPECIAL + i * P if i < NT else 0
                py = psumC.tile([P, 1024], dt.float32, tag="py", name=f"py{i}")
                for j in range(2):
                    for c in range(NC_):
                        nc.tensor.matmul(
                            py[:m, j * 512 : (j + 1) * 512],
                            oT_sb[:, c, mslice(i)],
                            wo_sb[:, c, j * 512 : (j + 1) * 512],
                            start=(c == 0),
                            stop=(c == NC_ - 1),
                        )
                y = sbC.tile([P, 1024], dt.float32, tag="y", bufs=2)
                nc.scalar.copy(y[:m, :], py[:m, :])
                nc.sync.dma_start(out[row0 : row0 + m, :], y[:m, :])

    nc.compile()
    return nc


def _get_compiled():
    global _COMPILED
    if _COMPILED is None:
        _COMPILED = _build()
    return _COMPILED


def _tile_cm(a2d, nchunks):
    """[K, F] -> [128, K//128, F] with element [p, c, f] = a2d[c*128+p, f]."""
    K, F = a2d.shape
    return np.ascontiguousarray(
        a2d.reshape(nchunks, P, F).transpose(1, 0, 2)
    )


def _prep(freqs_cos, freqs_sin, qkv_w, out_w, norm_q_w, norm_k_w):
    perm = np.concatenate([np.arange(SPECIAL, N), np.arange(0, SPECIAL)])
    wqkv_t = _tile_cm(np.asarray(qkv_w, np.float32).T.astype(bf16), NC_)
    wo_t = _tile_cm(np.asarray(out_w, np.float32).T.astype(bf16), NC_)

    c_r = np.asarray(freqs_cos, np.float32)[perm]  # [1032, 64] in m-order
    s_r = np.asarray(freqs_sin, np.float32)[perm]
    h2 = HD // 2

    def fold(w):
        w = np.asarray(w, np.float32)
        cw = c_r * w[None, :]
        sw = np.empty_like(s_r)
        sw[:, :h2] = -s_r[:, :h2] * w[None, h2:]
        sw[:, h2:] = s_r[:, h2:] * w[None, :h2]
        return cw, sw

    cq, sq_ = fold(norm_q_w)
    ck, sk_ = fold(norm_k_w)

    def padtab(t):
        tp = np.zeros(((NT + 1) * P, HD), np.float32)
        tp[:N] = t
        return _tile_cm(tp.astype(bf16), NT + 1)

    # additive mask canvases: 0 where attending, NEG elsewhere
    jj, ii = np.meshgrid(np.arange(P), np.arange(P), indexing="ij")
    mskc = np.full((2, P, CW), NEG, np.float32)
    for (H, s, ws, w, tlo, thi) in GEOM:
        for t in range(tlo, thi + 1):
            d = t - s
            ok = (np.abs(-4 * d + jj // GRID - ii // GRID) <= WINDOW) & (
                np.abs(jj % GRID - ii % GRID) <= WINDOW
            )
            mskc[H][:, ws + P * (t - tlo) : ws + P * (t - tlo) + P] = np.where(ok, 0.0, NEG)
    mskc[1][:, SPQ : SPQ + 64] = 0.0
    mskc[1][:SPECIAL, SPQ + 64 : SPQ + 72] = 0.0
    return dict(
        wqkv=wqkv_t,
        wo=wo_t,
        cosq=padtab(cq),
        sinq=padtab(sq_),
        cosk=padtab(ck),
        sink=padtab(sk_),
        mskc0=mskc[0].astype(bf16),
        mskc1=mskc[1].astype(bf16),
    )


def make_in_maps(hidden_states, freqs_cos, freqs_sin, qkv_w, out_w, norm_q_w, norm_k_w):
    shared = _prep(freqs_cos, freqs_sin, qkv_w, out_w, norm_q_w, norm_k_w)
    perm = np.concatenate([np.arange(SPECIAL, N), np.arange(0, SPECIAL)])
    hs = np.asarray(hidden_states, np.float32)
    in_maps = []
    for b in range(B):
        xb = hs[b][perm]                       # [1032, 1024] m-order
        xT = _tile_cm(np.ascontiguousarray(xb.T).astype(bf16), NC_)  # [128, 8, 1032]
        in_maps.append(dict(shared, xT=xT))
    return in_maps


def kernel(hidden_states, freqs_cos, freqs_sin, qkv_w, out_w, norm_q_w, norm_k_w):
    from concourse.bass_utils import run_bass_kernel_spmd

    nc = _get_compiled()
    in_maps = make_in_maps(
        hidden_states, freqs_cos, freqs_sin, qkv_w, out_w, norm_q_w, norm_k_w
    )
    res = run_bass_kernel_spmd(nc, in_maps, core_ids=list(range(B)))
    return np.stack([np.asarray(res.results[i]["out"], np.float32) for i in range(B)])


# revision 27
# speedup vs baseline: 1.0157x; 1.0157x over previous
"""Sparse 2D-sliding-window + global-token attention block on 8 TRN2 NeuronCores.

Strategy: data-parallel over batch (B=8 -> one batch element per core, zero
collectives). Per core, for one [1032, 1024] sequence:

  - tokens reordered host-side: 1024 patches first (8 exact tiles of 128 =
    4 grid rows each), 8 special/CLS tokens last.  Patch q-tile t only
    attends to patch k-tiles {t-1, t, t+1} plus the specials.
  - QKV projection in bf16 (lhsT = X^T tiles, rhs = W^T), RMS-norm + RoPE in
    row layout (norm weights folded into host-precomputed cos/sin tables),
    then PE-transpose of q~/k~ into [d, m] layout for the score matmuls.
  - scores computed transposed (S^T = K~ Q~^T) into two 3-bank PSUM
    "canvases" per head; the sparsity mask is baked in ADDITIVELY (0/-1e4)
    by an identity-weight matmul that also initializes each bank, so one
    wide scalar-engine Exp per canvas produces masked probabilities
    directly (softmax needs no max-subtraction: RMS-normed rows have L2
    norm exactly 8, so |s| <= 8 and exp(s/8) is safe).  V carries an
    appended ones-column so denominators fall out of the PV matmul as row
    64 of O^T; reciprocals are taken by the vector engine straight off
    PSUM row 64 and broadcast across partitions with a rank-1 matmul.
  - score/PV stages are software-pipelined across heads (scores of head
    h+1 are emitted before PV of head h) to keep the PE busy and the HAM
    clock-gate at full rate.
  - out-projection consumes O^T directly as lhsT (no O transpose needed).
"""

import numpy as np
import ml_dtypes

B, N, DIM, HEADS, HD = 8, 1032, 1024, 16, 64
SPECIAL, GRID, WINDOW = 8, 32, 3
NP = 1024          # patch tokens
P = 128
NT = NP // P       # 8 patch tiles (4 grid rows each)
NC_ = DIM // P     # 8 contraction chunks
EPS = 1e-6
NEG = -1.0e4       # additive mask value; exp(NEG/8) == 0 in bf16
CW = 1536          # canvas width (3 PSUM banks)
SPQ = 1408         # specials-q block offset within canvas half 1
bf16 = ml_dtypes.bfloat16

# ---- band geometry -------------------------------------------------------
# canvas half H in {0,1} holds k-tiles s = 4H..4H+3; window of k-tile s
# covers q-tiles t_lo..t_hi contiguously at canvas offset ws.
GEOM = []
for _H in (0, 1):
    _ws = 0
    for _s in range(4 * _H, 4 * _H + 4):
        _tlo, _thi = max(0, _s - 1), min(NT - 1, _s + 1)
        _w = P * (_thi - _tlo + 1)
        GEOM.append((_H, _s, _ws, _w, _tlo, _thi))
        _ws += _w

# score matmul pieces per half: (s, a, b, qa, stop) -> canvas[:, a:b] +=
# K_s^T Q[:, qa:qa+(b-a)]; `stop` marks the last accumulating matmul of a
# bank (bank2 of half 1 is closed later by the special-special matmul).
SCORE_PIECES = {0: [], 1: []}
for (_H, _s, _ws, _w, _tlo, _thi) in GEOM:
    _q0 = P * _tlo
    _a = _ws
    while _a < _ws + _w:
        _b = min(_ws + _w, (_a // 512 + 1) * 512)
        SCORE_PIECES[_H].append([_s, _a, _b, _q0 + (_a - _ws)])
        _a = _b
for _H in (0, 1):
    _last = {}
    for _idx, (_s, _a, _b, _qa) in enumerate(SCORE_PIECES[_H]):
        _last[_a // 512] = _idx
    for _bk, _idx in _last.items():
        _stop = not (_H == 1 and _bk == 2)  # bank2/half1 closed by ss matmul
        SCORE_PIECES[_H][_idx] = SCORE_PIECES[_H][_idx] + [_stop]
    for _p in SCORE_PIECES[_H]:
        if len(_p) == 4:
            _p.append(False)

# PV pieces per output bank b: (s, half, oa, ob, ra) ->
# po_b[:, oa:ob] += V_s^T P^T(canvas[half][:, ra:ra+(ob-oa)])
PV_PIECES = {0: [], 1: []}
for _b in (0, 1):
    for (_H, _s, _ws, _w, _tlo, _thi) in GEOM:
        _t0, _t1 = max(4 * _b, _tlo), min(4 * _b + 3, _thi)
        if _t0 > _t1:
            continue
        _oa = P * (_t0 - 4 * _b)
        _ob = P * (_t1 + 1 - 4 * _b)
        _ra = _ws + P * (_t0 - _tlo)
        PV_PIECES[_b].append((_s, _H, _oa, _ob, _ra))

_COMPILED = None


def _build():
    from contextlib import ExitStack
    import concourse.bass as bass
    import concourse.tile as tile
    from concourse import bacc, mybir
    from concourse.masks import make_identity

    dt = mybir.dt
    AF = mybir.ActivationFunctionType
    MUL = mybir.AluOpType.mult
    ADD = mybir.AluOpType.add

    nc = bacc.Bacc()

    xT = nc.declare_dram_parameter("xT", [P, NC_, N], dt.bfloat16, isOutput=False)
    wqkv = nc.declare_dram_parameter("wqkv", [P, NC_, 3 * DIM], dt.bfloat16, isOutput=False)
    wo = nc.declare_dram_parameter("wo", [P, NC_, DIM], dt.bfloat16, isOutput=False)
    # folded (norm-weight x cos/sin) tables, reordered to the m-layout, [128, 9, 64]
    cosq = nc.declare_dram_parameter("cosq", [P, NT + 1, HD], dt.bfloat16, isOutput=False)
    sinq = nc.declare_dram_parameter("sinq", [P, NT + 1, HD], dt.bfloat16, isOutput=False)
    cosk = nc.declare_dram_parameter("cosk", [P, NT + 1, HD], dt.bfloat16, isOutput=False)
    sink = nc.declare_dram_parameter("sink", [P, NT + 1, HD], dt.bfloat16, isOutput=False)
    mskc0 = nc.declare_dram_parameter("mskc0", [P, CW], dt.bfloat16, isOutput=False)
    mskc1 = nc.declare_dram_parameter("mskc1", [P, CW], dt.bfloat16, isOutput=False)
    out = nc.declare_dram_parameter("out", [N, DIM], dt.float32, isOutput=True)

    # m-tile geometry: tiles 0..7 are patches (128 rows), tile 8 is specials (8)
    def mslice(i):
        return slice(i * P, i * P + (P if i < NT else SPECIAL))

    def mp(i):
        return P if i < NT else SPECIAL

    with ExitStack() as ctx:
        ctx.enter_context(nc.allow_low_precision(reason="bf16 compute validated against f32 reference"))
        tc = ctx.enter_context(tile.TileContext(nc))
        persist = ctx.enter_context(tc.tile_pool(name="persist", bufs=1))

        # ---- resident SBUF tensors -------------------------------------
        ident = persist.tile([P, P], dt.bfloat16, tag="ident")
        make_identity(nc, ident[:])

        xT_sb = persist.tile([P, NC_, N], dt.bfloat16)
        wq_sb = persist.tile([P, NC_, 3 * DIM], dt.bfloat16)
        wo_sb = persist.tile([P, NC_, DIM], dt.bfloat16)
        tab = {}
        for nm in ("cosq", "sinq", "cosk", "sink"):
            tab[nm] = persist.tile([P, NT + 1, HD], dt.bfloat16, tag=f"tab_{nm}", name=f"tab_{nm}")
        msk_sb = [
            persist.tile([P, CW], dt.bfloat16, tag=f"mskc{_h}", name=f"mskc{_h}_sb")
            for _h in range(2)
        ]
        for c in range(NC_):
            nc.sync.dma_start(xT_sb[:, c, :], xT[:, c, :])
            nc.sync.dma_start(wq_sb[:, c, 0:1536], wqkv[:, c, 0:1536])
            nc.sync.dma_start(wq_sb[:, c, 1536:3072], wqkv[:, c, 1536:3072])
            if c == 3:
                for nm, ap in (("cosq", cosq), ("sinq", sinq), ("cosk", cosk), ("sink", sink)):
                    nc.sync.dma_start(tab[nm][:], ap[:])
        nc.sync.dma_start(msk_sb[0][:], mskc0[:])
        nc.sync.dma_start(msk_sb[1][:], mskc1[:])

        # q~^T stored one head per 128-partition slot with the other head's
        # 64 rows ZERO, so score matmuls contract over the full 128 partitions
        # (k=128 keeps PE-array utilization high -> HAM stays at full clock):
        # lhsT = kT_sb[:, ch, tile] holds the head PAIR, the zeros in qTz kill
        # the other head's contribution.
        qTz = persist.tile([P, HEADS, N], dt.bfloat16, tag="qTz")
        nc.gpsimd.memset(qTz[:], 0.0)
        kT_sb = persist.tile([P, NC_, N], dt.bfloat16, tag="kT")
        # normalized O^T overwrites kT_sb per head (kT for head pair (2c,2c+1)
        # is last read by scores of head 2c+1; norm trails by 2 pipeline
        # stages, so the overwrite is safe) -- saves 16.5KB/partition of SBUF
        oT_sb = kT_sb
        # V with an interleaved ones column: [128, 9 m-tiles, 16 heads, 65]
        v_sb = persist.tile([P, NT + 1, HEADS, HD + 1], dt.bfloat16, tag="v")
        nc.vector.memset(v_sb[:, :, :, HD : HD + 1], 1.0)

        eps_sb = persist.tile([P, 1], dt.float32, tag="eps")
        nc.vector.memset(eps_sb[:], EPS)

        # ---- phase A: QKV projection + RMS norm + RoPE + transpose -----
        with tc.tile_pool(name="psumA", bufs=2, space="PSUM") as psumA, \
             tc.tile_pool(name="sbufA", bufs=2) as sbA:
            # HAM warmup: keep the PE busy while the first DMAs land so the
            # clock-gate reaches 8/8 before the real matmuls start.
            warm = psumA.tile([P, 512], dt.float32, tag="tr", name="warm")
            for _w in range(36):
                nc.tensor.matmul(warm[:P, 0:P], ident[:], ident[:], start=True, stop=True)

            rope_pending = []

            def flush_transposes():
                for (ii, rope, which) in rope_pending:
                    mm = mp(ii)
                    mss = mslice(ii)
                    for half in (0, 1):
                        ptr = psumA.tile([P, 512], dt.bfloat16, tag="tr", name=f"tr{ii}_{half}")
                        for c2 in range(4):
                            cc = 4 * half + c2
                            nc.tensor.transpose(
                                ptr[:P, c2 * P : c2 * P + mm],
                                rope[:mm, cc * P : (cc + 1) * P],
                                ident[:mm, :mm],
                            )
                        src = ptr[:P, :].rearrange("p (c f) -> p c f", c=4)[:, :, :mm]
                        if which == "k":
                            nc.vector.tensor_copy(
                                kT_sb[:, 4 * half : 4 * half + 4, mss], src
                            )
                        else:
                            # d-chunk cc covers heads (2cc, 2cc+1): rows 0:64 of
                            # the transpose are head 2cc, rows 64:128 head 2cc+1
                            nc.vector.tensor_copy(
                                qTz[0:HD, 8 * half : 8 * half + 8 : 2, mss],
                                src[0:HD],
                            )
                            nc.vector.tensor_copy(
                                qTz[HD:P, 8 * half + 1 : 8 * half + 8 : 2, mss],
                                src[HD:P],
                            )
                rope_pending.clear()

            for i in [NT] + list(range(NT)):
                m = mp(i)
                ms = mslice(i)
                psA = psumA.tile([P, 1536], dt.float32, tag="qkv", name=f"psA{i}")
                psB = psumA.tile([P, 1536], dt.float32, tag="qkv", name=f"psB{i}")
                for c in range(NC_):
                    lhsT = xT_sb[:, c, ms]
                    for j in range(6):
                        pst = psA if j < 3 else psB
                        nc.tensor.matmul(
                            pst[:m, (j % 3) * 512 : (j % 3 + 1) * 512],
                            lhsT,
                            wq_sb[:, c, j * 512 : (j + 1) * 512],
                            start=(c == 0),
                            stop=(c == NC_ - 1),
                        )
                    if i == NT:
                        # fill the DMA-gated startup gaps with warmup matmuls
                        # so the HAM clock-gate stays at full rate
                        for _w in range(6):
                            nc.tensor.matmul(warm[:P, 0:P], ident[:], ident[:], start=True, stop=True)
                flush_transposes()
                # V: copy into interleaved [head, 65] layout (one wide ACT)
                nc.scalar.copy(
                    v_sb[:m, i, :, 0:HD],
                    psB[:m, 512:1536].rearrange("p (h d) -> p h d", h=HEADS),
                )
                # Q and K: norm + rope
                for which, (j0, cosn, sinn) in (
                    ("q", (0, "cosq", "sinq")),
                    ("k", (2, "cosk", "sink")),
                ):
                    raw = sbA.tile([P, DIM], dt.bfloat16, tag="raw")
                    if which == "q":
                        nc.scalar.copy(raw[:m, 0:1024], psA[:m, 0:1024])
                    else:
                        nc.scalar.copy(raw[:m, 0:512], psA[:m, 1024:1536])
                        nc.scalar.copy(raw[:m, 512:1024], psB[:m, 0:512])
                    sq = sbA.tile([P, DIM], dt.bfloat16, tag="tsin")
                    nc.gpsimd.tensor_tensor(sq[:m], raw[:m], raw[:m], op=MUL)
                    ssum = sbA.tile([P, HEADS], dt.float32, tag="ssum", bufs=3)
                    nc.vector.reduce_sum(
                        ssum[:m],
                        sq[:m].rearrange("p (h d) -> p h d", h=HEADS),
                        axis=mybir.AxisListType.X,
                    )
                    rstd = sbA.tile([P, HEADS], dt.float32, tag="rstd", bufs=3)
                    nc.scalar.activation(rstd[:m], ssum[:m], AF.Sqrt, bias=eps_sb[:m], scale=1.0 / HD)
                    rst = sbA.tile([P, HEADS], dt.bfloat16, tag="rst", bufs=3)
                    nc.vector.reciprocal(rst[:m], rstd[:m])
                    rv = raw[:m].rearrange("p (h two half) -> p h two half", h=HEADS, two=2)
                    cosw = tab[cosn][:m, i, None, :].to_broadcast((m, HEADS, HD))
                    tc_t = sbA.tile([P, DIM], dt.bfloat16, tag="tcos")
                    nc.vector.tensor_tensor(
                        tc_t[:m].rearrange("p (h d) -> p h d", h=HEADS),
                        raw[:m].rearrange("p (h d) -> p h d", h=HEADS),
                        cosw,
                        op=MUL,
                    )
                    ts_t = sbA.tile([P, DIM], dt.bfloat16, tag="tsin")
                    tsv = ts_t[:m].rearrange("p (h two half) -> p h two half", h=HEADS, two=2)
                    sin4 = (
                        tab[sinn][:m, i, None, :]
                        .rearrange("p o (two half) -> p o two half", two=2)
                        .to_broadcast((m, HEADS, 2, HD // 2))
                    )
                    nc.vector.tensor_tensor(tsv[:, :, :, :], rv[:, :, ::-1, :], sin4, op=MUL)
                    nc.vector.tensor_tensor(tc_t[:m], tc_t[:m], ts_t[:m], op=ADD)
                    rope = sbA.tile([P, DIM], dt.bfloat16, tag="rope", bufs=4)
                    nc.vector.tensor_tensor(
                        rope[:m].rearrange("p (h d) -> p h d", h=HEADS),
                        tc_t[:m].rearrange("p (h d) -> p h d", h=HEADS),
                        rst[:m, :, None].to_broadcast((m, HEADS, HD)),
                        op=MUL,
                    )
                    rope_pending.append((i, rope, which))

            flush_transposes()
            for c in range(NC_):
                nc.sync.dma_start(wo_sb[:, c, :], wo[:, c, :])

        # ---- phase B: banded attention, per head, software-pipelined ---
        with tc.tile_pool(name="psumB", bufs=2, space="PSUM") as psumB, \
             tc.tile_pool(name="sbufB", bufs=2) as sbB:

            def emit_scores(h):
                pb = HD * (h % 2)
                ch = h // 2
                qTh = qTz[:, h, :]          # [128, N], other head's rows zero
                kTh = kT_sb[:, ch, :]       # [128, N], head pair packed
                # special-k scores S^T[sk, q] = [8, 1024]
                spk = psumB.tile([P, 1024], dt.float32, tag="canvas", name=f"spk{h}")
                for jj in range(2):
                    nc.tensor.matmul(
                        spk[:SPECIAL, jj * 512 : (jj + 1) * 512],
                        kTh[:, NP : NP + SPECIAL],
                        qTh[:, jj * 512 : (jj + 1) * 512],
                        start=True,
                        stop=True,
                    )
                ptk = sbB.tile([P, 1024], dt.bfloat16, tag="ptk", bufs=2, name=f"ptk{h}")
                nc.scalar.activation(ptk[:SPECIAL, :], spk[:SPECIAL, :], AF.Exp, scale=0.125)
                ptses = []
                for half in (0, 1):
                    cv = psumB.tile([P, CW], dt.float32, tag="canvas", name=f"cv{h}_{half}")
                    # additive mask initializes all 3 banks (start=True)
                    for bk in range(3):
                        nc.tensor.matmul(
                            cv[:, bk * 512 : (bk + 1) * 512],
                            ident[:],
                            msk_sb[half][:, bk * 512 : (bk + 1) * 512],
                            start=True,
                            stop=False,
                            skip_group_check=True,
                        )
                    for (s, a, b, qa, is_stop) in SCORE_PIECES[half]:
                        nc.tensor.matmul(
                            cv[:, a:b],
                            kTh[:, s * P : (s + 1) * P],
                            qTh[:, qa : qa + (b - a)],
                            start=False,
                            stop=is_stop,
                            skip_group_check=True,
                        )
                    if half == 1:
                        for s in range(NT):
                            nc.tensor.matmul(
                                cv[:, SPQ + 8 * s : SPQ + 8 * s + 8],
                                kTh[:, s * P : (s + 1) * P],
                                qTh[:, NP : NP + SPECIAL],
                                start=False,
                                stop=False,
                                skip_group_check=True,
                            )
                        nc.tensor.matmul(
                            cv[:SPECIAL, SPQ + 64 : SPQ + 72],
                            kTh[:, NP : NP + SPECIAL],
                            qTh[:, NP : NP + SPECIAL],
                            start=False,
                            stop=True,
                            skip_group_check=True,
                        )
                    pts = sbB.tile([P, CW], dt.bfloat16, tag="pts", bufs=4, name=f"pts{h}_{half}")
                    nc.scalar.activation(pts[:], cv[:], AF.Exp, scale=0.125)
                    ptses.append(pts)
                return (h, ptses[0], ptses[1], ptk)

            def emit_pv(state):
                h, pts0, pts1, ptk = state
                pb = HD * (h % 2)
                ch = h // 2
                ptsh = (pts0, pts1)
                po = []
                for b2 in (0, 1):
                    pot = psumB.tile([P, 512], dt.float32, tag="pv", name=f"po{h}_{b2}")
                    # full-width specials matmul first: start=True covers the
                    # whole bank so the banded pieces accumulate on written psum
                    nc.tensor.matmul(
                        pot[: HD + 1, 0:512],
                        v_sb[:SPECIAL, NT, h, :],
                        ptk[:SPECIAL, b2 * 512 : (b2 + 1) * 512],
                        start=True,
                        stop=False,
                        skip_group_check=True,
                    )
                    for k2, (s, Hf, oa, ob, ra) in enumerate(PV_PIECES[b2]):
                        nc.tensor.matmul(
                            pot[: HD + 1, oa:ob],
                            v_sb[:, s, h, :],
                            ptsh[Hf][:, ra : ra + (ob - oa)],
                            start=False,
                            stop=(k2 == len(PV_PIECES[b2]) - 1),
                            skip_group_check=True,
                        )
                    po.append(pot)
                po8 = psumB.tile([P, 512], dt.float32, tag="pv", name=f"po8_{h}")
                for s in range(NT):
                    nc.tensor.matmul(
                        po8[: HD + 1, 0:SPECIAL],
                        v_sb[:, s, h, :],
                        pts1[:, SPQ + 8 * s : SPQ + 8 * s + 8],
                        start=(s == 0),
                        stop=False,
                    )
                nc.tensor.matmul(
                    po8[: HD + 1, 0:SPECIAL],
                    v_sb[:SPECIAL, NT, h, :],
                    pts1[:SPECIAL, SPQ + 64 : SPQ + 72],
                    start=False,
                    stop=True,
                )
                # denominators: gather row 64 of the po banks into the `rec`
                # row, DMA-scatter the 1024 band values across 128 partitions,
                # reciprocal on 128 DVE lanes, DMA back into the row.
                rec = sbB.tile([P, 1040], dt.bfloat16, tag="rec", bufs=2, name=f"rec{h}")
                dcol = sbB.tile([P, SPECIAL + 1], dt.bfloat16, tag="dcol", bufs=2, name=f"dc{h}")
                dcolr = sbB.tile([P, SPECIAL + 1], dt.bfloat16, tag="dcolr", bufs=2, name=f"dcr{h}")
                nc.vector.tensor_copy(rec[HD : HD + 1, 0:512], po[0][HD : HD + 1, 0:512])
                nc.vector.tensor_copy(rec[HD : HD + 1, 512:1024], po[1][HD : HD + 1, 0:512])
                nc.vector.tensor_copy(rec[HD : HD + 1, 1024:1032], po8[HD : HD + 1, 0:SPECIAL])
                nc.sync.dma_start(dcol[:, 0:SPECIAL], rec[HD : HD + 1, 0:1024])
                nc.sync.dma_start(dcol[0:SPECIAL, SPECIAL : SPECIAL + 1], rec[HD : HD + 1, 1024:1032])
                nc.vector.reciprocal(dcolr[:, 0:SPECIAL], dcol[:, 0:SPECIAL])
                nc.vector.reciprocal(
                    dcolr[0:SPECIAL, SPECIAL : SPECIAL + 1],
                    dcol[0:SPECIAL, SPECIAL : SPECIAL + 1],
                )
                # scatter 1/den back to partition 0 so gpsimd can broadcast it
                nc.sync.dma_start(rec[0:1, 0:1024], dcolr[:, 0:SPECIAL])
                nc.sync.dma_start(rec[0:1, 1024:1032], dcolr[0:SPECIAL, SPECIAL : SPECIAL + 1])
                ou = []
                for b2 in (0, 1):
                    out_t = sbB.tile([HD, 512], dt.bfloat16, tag="ou", bufs=4, name=f"ou{h}_{b2}")
                    nc.vector.tensor_copy(out_t[:HD, :], po[b2][:HD, 0:512])
                    ou.append(out_t)
                ou8 = sbB.tile([HD, SPECIAL], dt.bfloat16, tag="ou8", bufs=2, name=f"ou8_{h}")
                nc.vector.tensor_copy(ou8[:HD, :SPECIAL], po8[:HD, 0:SPECIAL])
                return (h, rec, ou[0], ou[1], ou8)

            def emit_norm(state):
                h, rec, ou0, ou1, ou8 = state
                pb = HD * (h % 2)
                ch = h // 2
                # broadcast 1/den from partition 0 to all 64 output partitions
                pbc = sbB.tile([HD, 1032], dt.bfloat16, tag="pbc", bufs=2, name=f"pbc{h}")
                nc.gpsimd.partition_broadcast(pbc[:HD, :], rec[0:1, 0:1032])
                for b2 in (0, 1):
                    nc.vector.tensor_tensor(
                        oT_sb[pb : pb + HD, ch, b2 * 512 : (b2 + 1) * 512],
                        (ou0, ou1)[b2][:HD, :],
                        pbc[:HD, b2 * 512 : (b2 + 1) * 512],
                        op=MUL,
                    )
                nc.vector.tensor_tensor(
                    oT_sb[pb : pb + HD, ch, NP : NP + SPECIAL],
                    ou8[:HD, :SPECIAL],
                    pbc[:HD, 1024:1032],
                    op=MUL,
                )

            stage1, stage2 = None, None
            for h in range(HEADS):
                cur = emit_scores(h)
                if stage1 is not None:
                    s2 = emit_pv(stage1)
                    if stage2 is not None:
                        emit_norm(stage2)
                    stage2 = s2
                stage1 = cur
            stage2_last = emit_pv(stage1)
            emit_norm(stage2)
            emit_norm(stage2_last)

        # ---- phase C: out projection -----------------------------------
        with tc.tile_pool(name="psumC", bufs=2, space="PSUM") as psumC, \
             tc.tile_pool(name="sbufC", bufs=2) as sbC:
            for i in [NT] + list(range(NT)):
                m = mp(i)
                row0 = SPECIAL + i * P if i < NT else 0
                py = psumC.tile([P, 1024], dt.float32, tag="py", name=f"py{i}")
                for j in range(2):
                    for c in range(NC_):
                        nc.tensor.matmul(
                            py[:m, j * 512 : (j + 1) * 512],
                            oT_sb[:, c, mslice(i)],
                            wo_sb[:, c, j * 512 : (j + 1) * 512],
                            start=(c == 0),
                            stop=(c == NC_ - 1),
                        )
                y = sbC.tile([P, 1024], dt.float32, tag="y", bufs=2)
                nc.scalar.copy(y[:m, :], py[:m, :])
                nc.sync.dma_start(out[row0 : row0 + m, :], y[:m, :])

    nc.compile()
    return nc


def _get_compiled():
    global _COMPILED
    if _COMPILED is None:
        _COMPILED = _build()
    return _COMPILED


def _tile_cm(a2d, nchunks):
    """[K, F] -> [128, K//128, F] with element [p, c, f] = a2d[c*128+p, f]."""
    K, F = a2d.shape
    return np.ascontiguousarray(
        a2d.reshape(nchunks, P, F).transpose(1, 0, 2)
    )


def _prep(freqs_cos, freqs_sin, qkv_w, out_w, norm_q_w, norm_k_w):
    perm = np.concatenate([np.arange(SPECIAL, N), np.arange(0, SPECIAL)])
    wqkv_t = _tile_cm(np.asarray(qkv_w, np.float32).T.astype(bf16), NC_)
    wo_t = _tile_cm(np.asarray(out_w, np.float32).T.astype(bf16), NC_)

    c_r = np.asarray(freqs_cos, np.float32)[perm]  # [1032, 64] in m-order
    s_r = np.asarray(freqs_sin, np.float32)[perm]
    h2 = HD // 2

    def fold(w):
        w = np.asarray(w, np.float32)
        cw = c_r * w[None, :]
        sw = np.empty_like(s_r)
        sw[:, :h2] = -s_r[:, :h2] * w[None, h2:]
        sw[:, h2:] = s_r[:, h2:] * w[None, :h2]
        return cw, sw

    cq, sq_ = fold(norm_q_w)
    ck, sk_ = fold(norm_k_w)

    def padtab(t):
        tp = np.zeros(((NT + 1) * P, HD), np.float32)
        tp[:N] = t
        return _tile_cm(tp.astype(bf16), NT + 1)

    # additive mask canvases: 0 where attending, NEG elsewhere
    jj, ii = np.meshgrid(np.arange(P), np.arange(P), indexing="ij")
    mskc = np.full((2, P, CW), NEG, np.float32)
    for (H, s, ws, w, tlo, thi) in GEOM:
        for t in range(tlo, thi + 1):
            d = t - s
            ok = (np.abs(-4 * d + jj // GRID - ii // GRID) <= WINDOW) & (
                np.abs(jj % GRID - ii % GRID) <= WINDOW
            )
            mskc[H][:, ws + P * (t - tlo) : ws + P * (t - tlo) + P] = np.where(ok, 0.0, NEG)
    mskc[1][:, SPQ : SPQ + 64] = 0.0
    mskc[1][:SPECIAL, SPQ + 64 : SPQ + 72] = 0.0
    return dict(
        wqkv=wqkv_t,
        wo=wo_t,
        cosq=padtab(cq),
        sinq=padtab(sq_),
        cosk=padtab(ck),
        sink=padtab(sk_),
        mskc0=mskc[0].astype(bf16),
        mskc1=mskc[1].astype(bf16),
    )


def make_in_maps(hidden_states, freqs_cos, freqs_sin, qkv_w, out_w, norm_q_w, norm_k_w):
    shared = _prep(freqs_cos, freqs_sin, qkv_w, out_w, norm_q_w, norm_k_w)
    perm = np.concatenate([np.arange(SPECIAL, N), np.arange(0, SPECIAL)])
    hs = np.asarray(hidden_states, np.float32)
    in_maps = []
    for b in range(B):
        xb = hs[b][perm]                       # [1032, 1024] m-order
        xT = _tile_cm(np.ascontiguousarray(xb.T).astype(bf16), NC_)  # [128, 8, 1032]
        in_maps.append(dict(shared, xT=xT))
    return in_maps


def kernel(hidden_states, freqs_cos, freqs_sin, qkv_w, out_w, norm_q_w, norm_k_w):
    from concourse.bass_utils import run_bass_kernel_spmd

    nc = _get_compiled()
    in_maps = make_in_maps(
        hidden_states, freqs_cos, freqs_sin, qkv_w, out_w, norm_q_w, norm_k_w
    )
    res = run_bass_kernel_spmd(nc, in_maps, core_ids=list(range(B)))
    return np.stack([np.asarray(res.results[i]["out"], np.float32) for i in range(B)])
